# revision 14
# baseline (speedup 1.0000x reference)
"""CategoricalLstmDecoder Trainium kernel (self-contained).

8-way gate-split tensor parallel across 8 NeuronCores; f32r matmuls;
h exchanged via AllGather; sigmoid via tanh identity.

Software-pipelined: independent matmul blocks (bias+recurrent parts)
are issued right after each AllGather trigger so the PE stays busy
during collective latency. Gate sums are split across two PSUM
accumulation groups (independent vs AG-dependent) and combined with a
vector add so the AG wait doesn't gate the independent block.

Layout notes (per core c of 8):
- Gate chunk order per core: [i(128), f(128), o(128), g(128)] rows, where
  i/f/o rows are pre-scaled by 0.5 host-side (sigmoid(x) = 0.5*tanh(x/2)+0.5).
- All matmuls f32r (fp32 storage, bf16-class speed).
- h exchanged as h.T chunks [128 hdim, 64 batch] via ncfw AllGather.
"""
import sys
sys.path.insert(0, "/opt/trn_rl_repo")
import numpy as np
import concourse.bass as bass
import concourse.tile as tile
from concourse import bacc, mybir

N = 8
B = 64
HID = 1024
VOCAB = 512
CH = HID // N  # 128 h-dims per core
GW = 4 * CH    # 512 gate rows per core
f32 = mybir.dt.float32
f32r = mybir.dt.float32r
AFT = mybir.ActivationFunctionType
ALU = mybir.AluOpType


def host_prep(z, fc_w, fc_b, w_ih0, w_hh0, b_ih0, b_hh0,
              w_ih1, w_hh1, b_ih1, b_hh1, out_w, out_b):
    """Build per-core input dicts. All fp32 numpy."""
    z = np.asarray(z, np.float32)
    h_init = np.tanh(z @ np.asarray(fc_w, np.float32).T + np.asarray(fc_b, np.float32))  # [B, HID]
    # hT layout [128, 8, 64]: [p, j, b] = h_init[b, 128*j + p]
    hT_init = np.transpose(h_init.reshape(B, N, CH), (2, 1, 0)).copy()  # [CH, N, B]

    b0 = np.asarray(b_ih0, np.float32) + np.asarray(b_hh0, np.float32)
    b1 = np.asarray(b_ih1, np.float32) + np.asarray(b_hh1, np.float32)

    def chunk_rows(c):
        # rows of the 4H gate matrix owned by core c, in [i, f, o, g] order
        i0 = np.arange(c * CH, (c + 1) * CH)
        return np.concatenate([i0, HID + i0, 3 * HID + i0, 2 * HID + i0])

    def prep_w(w, c):
        # returns W.T chunk [K, GW] with i/f/o halved, as [128, K//128, GW]
        w = np.asarray(w, np.float32)
        rows = w[chunk_rows(c)]  # [GW, K]
        sgn = np.ones((GW, 1), np.float32)
        sgn[: 3 * CH] = 0.5
        rows = rows * sgn
        K = rows.shape[1]
        return np.ascontiguousarray(
            rows.T.reshape(K // 128, 128, GW).transpose(1, 0, 2))  # [128, K/128, GW]

    def prep_b(b, c):
        bb = b[chunk_rows(c)].astype(np.float32).copy()
        bb[: 3 * CH] *= 0.5
        return bb.reshape(1, GW)

    out_wT = np.ascontiguousarray(
        np.asarray(out_w, np.float32).T.reshape(N, CH, VOCAB).transpose(1, 0, 2))  # [128, 8, 512]

    in_maps = []
    for c in range(N):
        in_maps.append({
            "hT_init": hT_init,
            "wih0": prep_w(w_ih0, c),
            "whh0": prep_w(w_hh0, c),
            "wih1": prep_w(w_ih1, c),
            "whh1": prep_w(w_hh1, c),
            "outw": out_wT,
            "b0": prep_b(b0, c),
            "b1": prep_b(b1, c),
            "outb": np.asarray(out_b, np.float32).reshape(1, VOCAB).copy(),
            "ones": np.ones((1, B), np.float32),
            "eye": np.eye(B, dtype=np.float32),
        })
    return in_maps


def build_kernel(T):
    nc = bacc.Bacc("TRN2", target_bir_lowering=False, debug=False, num_devices=N)
    dp = nc.declare_dram_parameter
    hT_init_d = dp("hT_init", [CH, N, B], f32, isOutput=False)
    wih0_d = dp("wih0", [128, 4, GW], f32, isOutput=False)
    whh0_d = dp("whh0", [128, 8, GW], f32, isOutput=False)
    wih1_d = dp("wih1", [128, 8, GW], f32, isOutput=False)
    whh1_d = dp("whh1", [128, 8, GW], f32, isOutput=False)
    outw_d = dp("outw", [128, 8, VOCAB], f32, isOutput=False)
    b0_d = dp("b0", [1, GW], f32, isOutput=False)
    b1_d = dp("b1", [1, GW], f32, isOutput=False)
    outb_d = dp("outb", [1, VOCAB], f32, isOutput=False)
    ones_d = dp("ones", [1, B], f32, isOutput=False)
    eye_d = dp("eye", [B, B], f32, isOutput=False)
    out_d = dp("out", [T, B, VOCAB], f32, isOutput=True)

    with tile.TileContext(nc) as tc:
        with (
            tc.tile_pool(name="wpool", bufs=1) as wpool,
            tc.tile_pool(name="state", bufs=1) as state,
            tc.tile_pool(name="sp", bufs=2) as sp,
            tc.tile_pool(name="ps", bufs=1, space=bass.MemorySpace.PSUM) as ps,
            tc.tile_pool(name="dram", bufs=2, space="DRAM") as dram,
        ):
            # ---- load + round weights (one-time) ----
            def load_round(dram_t, shape, name):
                stage = sp.tile(shape, f32, tag="wstage")
                nc.sync.dma_start(stage[:], dram_t[:])
                wr = wpool.tile(shape, f32r, tag=f"w_{name}")
                nc.vector.tensor_copy(wr[:], stage[:])
                return wr

            wih0 = load_round(wih0_d, [128, 4, GW], "wih0")
            whh0 = load_round(whh0_d, [128, 8, GW], "whh0")
            wih1 = load_round(wih1_d, [128, 8, GW], "wih1")
            whh1 = load_round(whh1_d, [128, 8, GW], "whh1")
            outw = load_round(outw_d, [128, 8, VOCAB], "outw")
            b0 = load_round(b0_d, [1, GW], "b0")
            b1 = load_round(b1_d, [1, GW], "b1")
            outb = load_round(outb_d, [1, VOCAB], "outb")
            ones = load_round(ones_d, [1, B], "ones")
            eye = wpool.tile([B, B], f32, tag="eye")
            nc.sync.dma_start(eye[:], eye_d[:])
            hTi = load_round(hT_init_d, [CH, N, B], "hTi")  # gathered h0.T (f32r)

            c0 = state.tile([B, CH], f32)
            c1 = state.tile([B, CH], f32)
            nc.vector.memset(c0[:], 0.0)
            nc.vector.memset(c1[:], 0.0)

            def mk_gather(li):
                """DRAM tiles for one exchange."""
                cin = dram.tile([CH, B], f32, tag=f"cin{li}")
                gout = dram.tile([N * CH, B], f32, tag=f"gout{li}",
                                 addr_space="Shared")
                return cin, gout

            def exchange_start(li, hT):
                """sbuf hT [CH, B] -> DMA -> AllGather trigger."""
                cin, gout = mk_gather(li)
                nc.sync.dma_start(cin[:], hT[:])
                nc.gpsimd.collective_compute(
                    "AllGather", ALU.bypass,
                    replica_groups=[list(range(N))],
                    ins=[cin.opt()], outs=[gout.opt()],
                )
                return gout

            def exchange_finish(li, gout):
                """DMA gathered chunks back into sbuf [CH, N, B] (f32)."""
                hg = sp.tile([CH, N, B], f32, tag=f"hg{li}")
                src = gout.opt().rearrange("(c p) b -> p c b", p=CH)
                nc.sync.dma_start(hg[:, 0:4, :], src[:, 0:4, :])
                nc.gpsimd.dma_start(hg[:, 4:8, :], src[:, 4:8, :])
                return hg

            def lstm_front(li, gin, cstate):
                """gin [B, GW] -> (sg tile, nothing); issues tanh+sig ops."""
                th = sp.tile([B, GW], f32, tag=f"th{li}")
                nc.scalar.activation(th[:], gin, AFT.Tanh)
                sg = sp.tile([B, 3 * CH], f32, tag=f"sg{li}")
                nc.vector.tensor_scalar(sg[:], th[:, 0:3 * CH], 0.5, 0.5,
                                        ALU.mult, ALU.add)
                return th, sg

            def lstm_back(li, th, sg, cstate, zT):
                """c update + transposed h production. Returns hT sbuf [CH, B]."""
                tg = th[:, 3 * CH:GW]
                t1 = sp.tile([B, CH], f32, tag=f"t1{li}")
                nc.vector.tensor_mul(t1[:], sg[:, 0:CH], tg)              # sig(i)*tanh(g)
                nc.gpsimd.tensor_mul(cstate[:], cstate[:], sg[:, CH:2 * CH])  # c *= sig(f)
                nc.vector.tensor_add(cstate[:], cstate[:], t1[:])
                tc_ = sp.tile([B, CH], f32, tag=f"tc{li}")
                nc.scalar.activation(tc_[:], cstate[:], AFT.Tanh)
                h = sp.tile([B, CH], f32, tag=f"h{li}")
                nc.vector.tensor_mul(h[:], sg[:, 2 * CH:3 * CH], tc_[:])  # sig(o)*tanh(c)
                nc.tensor.transpose(zT[:], h[:], eye[:])
                hTs = sp.tile([CH, B], f32, tag=f"hTs{li}")
                nc.vector.tensor_copy(hTs[:], zT[:])
                return hTs

            R = lambda ap: ap.bitcast(f32r)

            # ---- prologue: g0a(0) = b0 + whh0 @ hTi ----
            g0a = ps.tile([B, GW], f32, tag="g0a")
            nc.tensor.matmul(g0a[:], ones[:], b0[:], start=True, stop=False)
            for j in range(8):
                nc.tensor.matmul(g0a[:], hTi[:, j, :], whh0[:, j, :],
                                 start=False, stop=(j == 7))
            hT1g = hTi          # full h1.T for whh1 (init: h_init)
            zT0 = ps.tile([CH, B], f32, tag="zT0")
            zT1 = ps.tile([CH, B], f32, tag="zT1")

            for t in range(T):
                th0, sg0 = lstm_front(0, g0a[:], c0)

                # ---- g1 group: independent part issued early (bias + whh1) ----
                g1 = ps.tile([B, GW], f32, tag="g1")
                nc.tensor.matmul(g1[:], ones[:], b1[:], start=True, stop=False)
                for j in range(8):
                    nc.tensor.matmul(g1[:], hT1g[:, j, :], whh1[:, j, :],
                                     start=False, stop=False)
                hT0 = lstm_back(0, th0, sg0, c0, zT0)
                gout0 = exchange_start(0, hT0)
                hg0 = exchange_finish(0, gout0)

                # ---- AG0-dependent tail: wih1 @ h0(t) ----
                for j in range(8):
                    nc.tensor.matmul(g1[:], R(hg0)[:, j, :], wih1[:, j, :],
                                     start=False, stop=(j == 7))

                # ---- lstm1 ----
                th1, sg1 = lstm_front(1, g1[:], c1)
                # ---- g0(t+1) group: independent part (bias + whh0 @ h0(t)) ----
                if t + 1 < T:
                    g0a = ps.tile([B, GW], f32, tag="g0a")
                    nc.tensor.matmul(g0a[:], ones[:], b0[:], start=True, stop=False)
                    for j in range(8):
                        nc.tensor.matmul(g0a[:], R(hg0)[:, j, :], whh0[:, j, :],
                                         start=False, stop=False)
                hT1 = lstm_back(1, th1, sg1, c1, zT1)
                gout1 = exchange_start(1, hT1)
                hg1 = exchange_finish(1, gout1)
                hT1g = R(hg1)

                # ---- logits (AG1-dependent) ----
                lg = ps.tile([B, VOCAB], f32, tag="lg")
                nc.tensor.matmul(lg[:], ones[:], outb[:], start=True, stop=False)
                for j in range(8):
                    nc.tensor.matmul(lg[:], R(hg1)[:, j, :], outw[:, j, :],
                                     start=False, stop=(j == 7))
                # ---- softmax -> xT -> close g0(t+1) group with wih0 ----
                if t + 1 < T:
                    ex = sp.tile([B, VOCAB], f32r, tag="ex")
                    sums = sp.tile([B, 1], f32, tag="sums")
                    nc.scalar.activation(ex[:], lg[:], AFT.Exp, accum_out=sums[:])
                lgs = sp.tile([B, VOCAB], f32, tag="lgs")
                nc.scalar.copy(lgs[:], lg[:])
                nc.gpsimd.dma_start(out_d[t], lgs[:])
                if t + 1 < T:
                    rr = sp.tile([B, 1], f32, tag="rr")
                    nc.vector.reciprocal(rr[:], sums[:])
                    dg = sp.tile([B, B], f32r, tag="dg")
                    nc.vector.tensor_scalar_mul(dg[:], eye[:], rr[:])
                    zxT = ps.tile([128, 4, B], f32, tag="zxT")
                    for j in range(4):
                        nc.tensor.matmul(zxT[:, j, :], ex[:, 128 * j:128 * (j + 1)],
                                         dg[:], start=True, stop=True)
                    xT = sp.tile([128, 4, B], f32r, tag="xT")
                    nc.vector.tensor_copy(xT[:], zxT[:])
                    for j in range(4):
                        nc.tensor.matmul(g0a[:], xT[:, j, :], wih0[:, j, :],
                                         start=False, stop=(j == 3))

    nc.compile()
    return nc


T_STEPS = 512


def postprocess(res):
    """BassKernelResults -> full logits [T, B, V] fp32."""
    return np.asarray(res.results[0]["out"], np.float32)


def kernel(**inputs):
    import numpy as np
    from concourse import bass_utils
    in_maps = host_prep(**{k: np.asarray(v) for k, v in inputs.items()})
    nc = build_kernel(T_STEPS)
    res = bass_utils.run_bass_kernel_spmd(
        nc, in_maps, core_ids=list(range(N)), trace=False)
    logits = postprocess(res)  # [T, B, V] fp32
    return np.ascontiguousarray(
        np.transpose(logits, (1, 0, 2))[:, :, None, :].astype(np.float32))
